# revision 17
# baseline (speedup 1.0000x reference)
"""Trainium2 Bass kernel for nn_ExLoss (scatter_memory).

Computation (reference):
    index_list = concat([positive_index, negative_index], axis=1)   # [N, K]
    logits[n, k] = inputs[n] . M[index_list[n, k]] * T              # [N, K]
    logp = log_softmax(logits, axis=1)
    loss = -sum_j cof[j] * mean_n logp[n, j]   (j < Kp)
    return (loss, logits)

Shapes: N=128, C=2048, NC=50000, Kp=4, Kn=508, K=512, T=1.0.

Strategy (8 NeuronCores, raw Bass — the walrus build in this container
rejects Tile's multi-wait sync encoding, so all waits are standalone):
  * Data-parallel over the batch: 16 rows of `inputs` per core; M (400 MB)
    replicated into every core's HBM.
  * Per (row n, k-tile j): one indirect DMA gathers 128 rows of M (8 KB
    contiguous each, one per SBUF partition) -> [128, 2048] tile.
    64 gathers/core = 64 MB of HBM traffic (the memory-roofline term,
    ~180 us at ~360 GB/s).
  * TensorE replicates x_n across partitions via one-hot matmul into PSUM
    (double-buffered).
  * VectorE computes products g * xrep (f32, one pass per tile).
    Row-sum reduction is split between ScalarE (activation Copy with
    accum_out) and VectorE (tensor_reduce) so neither exceeds the DMA
    bottleneck.
  * Logits land as [128, 64] per core (partition p = k%128, column =
    n*4 + k//128); the host reassembles [128, 512] and computes the tiny
    log-softmax + weighted loss in f32 numpy.
"""

import contextlib
import os

import numpy as np

import concourse.bass as bass
import concourse.mybir as mybir
from concourse.bass_utils import run_bass_kernel_spmd

# Problem constants (hardcoded per contest contract).
N = 128
C = 2048
NROWS = 50000
KP = 4
KN = 508
K = KP + KN  # 512
T = 1.0
NCORES = 8
NPC = N // NCORES  # 16 batch rows per core
JT = K // 128  # 4 k-tiles of 128 indices per batch row
NT = NPC * JT  # 64 gather tiles / logits columns per core

GB = int(os.environ.get("KERNEL_GB", "12"))  # gather buffers (1 MB tiles)
SB = int(os.environ.get("KERNEL_SB", "4"))  # scratch (product) buffers
# every RDVE-th reduction runs on VectorE, the rest on ScalarE
RDVE = int(os.environ.get("KERNEL_RDVE", "8"))

_LAST_RESULTS = None  # BassKernelResults of the last run (for test.py)


def _reducer_is_dve(t):
    # the last tile reduces on VectorE: it is already on the critical tail
    # right after the final product, avoiding the ScalarE handoff latency
    return t == NT - 1 or t % RDVE == RDVE - 1


def build_nc():
    """Build the per-core Bass program (SPMD: same program on all 8 cores)."""
    nc = bass.Bass("TRN2", target_bir_lowering=False, debug=False)

    xoh_d = nc.dram_tensor("xoh", [NPC, C + NPC * 128], mybir.dt.float32,
                           kind="ExternalInput")
    idx_d = nc.dram_tensor("midx", [128, NT], mybir.dt.int32,
                           kind="ExternalInput")
    m_d = nc.dram_tensor("mbank", [NROWS, C], mybir.dt.float32,
                         kind="ExternalInput")
    out_d = nc.dram_tensor("logits_t", [128, NT], mybir.dt.float32,
                           kind="ExternalOutput")

    n_act = sum(0 if _reducer_is_dve(t) else 1 for t in range(NT))
    n_dve = NT - n_act
    # act_count[t] = number of ACT reductions with t' <= t (same for DVE)
    act_count = np.cumsum([0 if _reducer_is_dve(t) else 1 for t in range(NT)])
    dve_count = np.cumsum([1 if _reducer_is_dve(t) else 0 for t in range(NT)])

    with (
        contextlib.ExitStack() as ctx,
        nc.sbuf_tensor("idx_t", [128, NT], mybir.dt.int32) as idx_t,
        nc.sbuf_tensor("xoh_t", [NPC, C + NPC * 128], mybir.dt.float32) as xoh_t,
        nc.sbuf_tensor("g_t", [128, GB * C], mybir.dt.float32) as g_t,
        nc.sbuf_tensor("sc_t", [128, SB * C], mybir.dt.float32) as sc_t,
        nc.sbuf_tensor("lg_t", [128, NT], mybir.dt.float32) as lg_t,
        nc.psum_tensor("xrep0", [128, C], mybir.dt.float32) as xrep0,
        nc.psum_tensor("xrep1", [128, C], mybir.dt.float32) as xrep1,
        nc.semaphore("sem_idx") as sem_idx,
        nc.semaphore("sem_xoh") as sem_xoh,

        nc.semaphore("sem_x") as sem_x,
        nc.semaphore("sem_mul") as sem_mul,
        nc.semaphore("sem_ra") as sem_ra,
        nc.semaphore("sem_rd") as sem_rd,
        nc.semaphore("sem_out") as sem_out,
        nc.Block() as block,
    ):
        xrep = [xrep0, xrep1]
        sem_g = [ctx.enter_context(nc.semaphore(f"sem_g{i}"))
                 for i in range(GB)]

        @block.sync
        def _(sync):
            sync.dma_start(out=idx_t[:], in_=idx_d.ap()).then_inc(sem_idx, 16)
            sync.dma_start(out=xoh_t[:], in_=xoh_d.ap()).then_inc(sem_xoh, 16)
            if n_act:
                sync.wait_ge(sem_ra, n_act)
            if n_dve:
                sync.wait_ge(sem_rd, n_dve)
            sync.dma_start(out=out_d.ap(), in_=lg_t[:]).then_inc(sem_out, 16)
            sync.wait_ge(sem_out, 16)

        @block.tensor
        def _(tensor):
            tensor.wait_ge(sem_xoh, 16)
            for n in range(NPC):
                if n >= 2:
                    # PSUM slot n%2 free once all 4 products of row n-2 read it
                    tensor.wait_ge(sem_mul, 4 * n - 4)
                for jj in range(C // 512):
                    nc.tensor.matmul(
                        out=xrep[n % 2][:, jj * 512:(jj + 1) * 512],
                        lhsT=xoh_t[:, C + n * 128:C + (n + 1) * 128],
                        rhs=xoh_t[:, jj * 512:(jj + 1) * 512],
                        start=True, stop=True,
                    ).then_inc(sem_x, 1)

        @block.gpsimd
        def _(gpsimd):
            # NOTE: the HW indirect DMA consumes ONE offset per partition
            # (out[p, :] streams contiguously from M[offs[p]]), so each call
            # gathers exactly 128 rows. Per-slot semaphores + the sem_mul
            # throttle keep <=1 call in flight per semaphore so its 16
            # per-engine increments can't interleave with another call's.
            gpsimd.wait_ge(sem_idx, 16)
            for t in range(NT):
                if t >= GB:
                    gpsimd.wait_ge(sem_mul, t - GB + 1)
                gpsimd.indirect_dma_start(
                    out=g_t[:, (t % GB) * C:(t % GB + 1) * C],
                    out_offset=None,
                    in_=m_d.ap(),
                    in_offset=bass.IndirectOffsetOnAxis(
                        ap=idx_t[:, t:t + 1], axis=0),
                ).then_inc(sem_g[t % GB], 16)

        @block.vector
        def _(vector):
            for t in range(NT):
                n = t // 4
                if t % 4 == 0:
                    vector.wait_ge(sem_x, 4 * (n + 1))
                vector.wait_ge(sem_g[t % GB], 16 * (t // GB + 1))
                if t >= SB:
                    # scratch slot free once reduction t-SB retired
                    if _reducer_is_dve(t - SB):
                        vector.wait_ge(sem_rd, int(dve_count[t - SB]))
                    else:
                        vector.wait_ge(sem_ra, int(act_count[t - SB]))
                sc = sc_t[:, (t % SB) * C:(t % SB + 1) * C]
                nc.vector.tensor_tensor(
                    out=sc, in0=g_t[:, (t % GB) * C:(t % GB + 1) * C],
                    in1=xrep[n % 2][:],
                    op=mybir.AluOpType.mult,
                ).then_inc(sem_mul, 1)
                if _reducer_is_dve(t):
                    # self-wait: the engine pipeline may not have drained the
                    # product write yet
                    vector.wait_ge(sem_mul, t + 1)
                    nc.vector.tensor_reduce(
                        out=lg_t[:, t:t + 1], in_=sc,
                        axis=mybir.AxisListType.X,
                        op=mybir.AluOpType.add,
                    ).then_inc(sem_rd, 1)

        @block.scalar
        def _(scalar):
            for t in range(NT):
                if _reducer_is_dve(t):
                    continue
                scalar.wait_ge(sem_mul, t + 1)
                sc = sc_t[:, (t % SB) * C:(t % SB + 1) * C]
                # in-place: the mandatory elementwise out overwrites the
                # products we no longer need; only accum_out matters.
                nc.scalar.activation(
                    out=sc, in_=sc,
                    func=mybir.ActivationFunctionType.Copy,
                    accum_out=lg_t[:, t:t + 1],
                ).then_inc(sem_ra, 1)

    return nc


def _make_onehot():
    oh = np.zeros((NPC, NPC, 128), dtype=np.float32)
    for n in range(NPC):
        oh[n, n, :] = 1.0
    return oh.reshape(NPC, NPC * 128)


def make_in_maps(inputs, positive_index, negative_index, M):
    """Shard the full inputs into per-core input dicts."""
    index_list = np.concatenate(
        [np.asarray(positive_index), np.asarray(negative_index)], axis=1
    ).astype(np.int32)  # [N, K]
    inputs = np.ascontiguousarray(np.asarray(inputs, dtype=np.float32))
    M = np.ascontiguousarray(np.asarray(M, dtype=np.float32))
    oh = _make_onehot()

    in_maps = []
    for c in range(NCORES):
        idx_c = index_list[c * NPC:(c + 1) * NPC]  # [16, 512]
        # midx[p, n*JT + j] = idx_c[n, j*128 + p]
        midx = np.ascontiguousarray(
            idx_c.reshape(NPC, JT, 128).transpose(2, 0, 1).reshape(128, NT))
        in_maps.append({
            "xoh": np.ascontiguousarray(
                np.concatenate([inputs[c * NPC:(c + 1) * NPC], oh], axis=1)),
            "midx": midx,
            "mbank": M,
        })
    return in_maps


def assemble_logits(per_core_outs):
    """[128, 64] per core -> full [128, 512] logits."""
    shards = []
    for c in range(NCORES):
        out_c = per_core_outs[c]  # [128, NT]
        # logits[n, j*128+p] = out_c[p, n*JT+j]
        shard = out_c.reshape(128, NPC, JT).transpose(1, 2, 0).reshape(NPC, K)
        shards.append(shard)
    return np.concatenate(shards, axis=0).astype(np.float32)


def _loss_from_logits(logits, cof):
    logits = logits.astype(np.float32)
    m = logits.max(axis=1, keepdims=True)
    lse = m + np.log(np.exp(logits - m).sum(axis=1, keepdims=True))
    logp = logits - lse  # [N, K]
    loss = -(np.asarray(cof, dtype=np.float32) * logp[:, :KP].mean(axis=0)).sum()
    return np.float32(loss)


def kernel(inputs, positive_index, negative_index, cof, M):
    global _LAST_RESULTS
    in_maps = make_in_maps(inputs, positive_index, negative_index, M)
    nc = build_nc()
    trace = os.environ.get("KERNEL_TRACE", "0") == "1"
    res = run_bass_kernel_spmd(
        nc, in_maps, core_ids=list(range(NCORES)), trace=trace)
    _LAST_RESULTS = res
    logits = assemble_logits([r["logits_t"] for r in res.results])
    loss = _loss_from_logits(logits, cof)
    return loss, logits


# revision 18
# speedup vs baseline: 1.0026x; 1.0026x over previous
"""Trainium2 Bass kernel for nn_ExLoss (scatter_memory).

Computation (reference):
    index_list = concat([positive_index, negative_index], axis=1)   # [N, K]
    logits[n, k] = inputs[n] . M[index_list[n, k]] * T              # [N, K]
    logp = log_softmax(logits, axis=1)
    loss = -sum_j cof[j] * mean_n logp[n, j]   (j < Kp)
    return (loss, logits)

Shapes: N=128, C=2048, NC=50000, Kp=4, Kn=508, K=512, T=1.0.

Strategy (8 NeuronCores, raw Bass — the walrus build in this container
rejects Tile's multi-wait sync encoding, so all waits are standalone):
  * Data-parallel over the batch: 16 rows of `inputs` per core; M (400 MB)
    replicated into every core's HBM.
  * Per (row n, k-tile j): one indirect DMA gathers 128 rows of M (8 KB
    contiguous each, one per SBUF partition) -> [128, 2048] tile.
    64 gathers/core = 64 MB of HBM traffic (the memory-roofline term,
    ~180 us at ~360 GB/s).
  * TensorE replicates x_n across partitions via one-hot matmul into PSUM
    (double-buffered).
  * VectorE computes products g * xrep (f32, one pass per tile).
    Row-sum reduction is split between ScalarE (activation Copy with
    accum_out) and VectorE (tensor_reduce) so neither exceeds the DMA
    bottleneck.
  * Logits land as [128, 64] per core (partition p = k%128, column =
    n*4 + k//128); the host reassembles [128, 512] and computes the tiny
    log-softmax + weighted loss in f32 numpy.
"""

import contextlib
import os

import numpy as np

import concourse.bass as bass
import concourse.mybir as mybir
from concourse.bass_utils import run_bass_kernel_spmd

# Problem constants (hardcoded per contest contract).
N = 128
C = 2048
NROWS = 50000
KP = 4
KN = 508
K = KP + KN  # 512
T = 1.0
NCORES = 8
NPC = N // NCORES  # 16 batch rows per core
JT = K // 128  # 4 k-tiles of 128 indices per batch row
NT = NPC * JT  # 64 gather tiles / logits columns per core

GB = int(os.environ.get("KERNEL_GB", "12"))  # gather buffers (1 MB tiles)
SB = int(os.environ.get("KERNEL_SB", "4"))  # scratch (product) buffers
# every RDVE-th reduction runs on VectorE, the rest on ScalarE
RDVE = int(os.environ.get("KERNEL_RDVE", "8"))

_LAST_RESULTS = None  # BassKernelResults of the last run (for test.py)


def _reducer_is_dve(t):
    # the last tile reduces on VectorE: it is already on the critical tail
    # right after the final product, avoiding the ScalarE handoff latency
    return t == NT - 1 or t % RDVE == RDVE - 1


def build_nc():
    """Build the per-core Bass program (SPMD: same program on all 8 cores)."""
    nc = bass.Bass("TRN2", target_bir_lowering=False, debug=False)

    xoh_d = nc.dram_tensor("xoh", [NPC, C + NPC * 128], mybir.dt.float32,
                           kind="ExternalInput")
    idx_d = nc.dram_tensor("midx", [128, NT], mybir.dt.int32,
                           kind="ExternalInput")
    m_d = nc.dram_tensor("mbank", [NROWS, C], mybir.dt.float32,
                         kind="ExternalInput")
    out_d = nc.dram_tensor("logits_t", [128, NT], mybir.dt.float32,
                           kind="ExternalOutput")

    n_act = sum(0 if _reducer_is_dve(t) else 1 for t in range(NT))
    n_dve = NT - n_act
    # act_count[t] = number of ACT reductions with t' <= t (same for DVE)
    act_count = np.cumsum([0 if _reducer_is_dve(t) else 1 for t in range(NT)])
    dve_count = np.cumsum([1 if _reducer_is_dve(t) else 0 for t in range(NT)])

    with (
        contextlib.ExitStack() as ctx,
        nc.sbuf_tensor("idx_t", [128, NT], mybir.dt.int32) as idx_t,
        nc.sbuf_tensor("xoh_t", [NPC, C + NPC * 128], mybir.dt.float32) as xoh_t,
        nc.sbuf_tensor("g_t", [128, GB * C], mybir.dt.float32) as g_t,
        nc.sbuf_tensor("sc_t", [128, SB * C], mybir.dt.float32) as sc_t,
        nc.sbuf_tensor("lg_t", [128, NT], mybir.dt.float32) as lg_t,
        nc.psum_tensor("xrep0", [128, C], mybir.dt.float32) as xrep0,
        nc.psum_tensor("xrep1", [128, C], mybir.dt.float32) as xrep1,
        nc.semaphore("sem_idx") as sem_idx,
        nc.semaphore("sem_xoh") as sem_xoh,

        nc.semaphore("sem_x") as sem_x,
        nc.semaphore("sem_mul") as sem_mul,
        nc.semaphore("sem_ra") as sem_ra,
        nc.semaphore("sem_rd") as sem_rd,
        nc.semaphore("sem_gl") as sem_gl,
        nc.semaphore("sem_out") as sem_out,
        nc.Block() as block,
    ):
        xrep = [xrep0, xrep1]
        sem_g = [ctx.enter_context(nc.semaphore(f"sem_g{i}"))
                 for i in range(GB)]

        @block.sync
        def _(sync):
            sync.dma_start(out=idx_t[:], in_=idx_d.ap()).then_inc(sem_idx, 16)
            sync.dma_start(out=xoh_t[:], in_=xoh_d.ap()).then_inc(sem_xoh, 16)
            if n_act:
                sync.wait_ge(sem_ra, n_act)
            if n_dve:
                sync.wait_ge(sem_rd, n_dve)
            sync.dma_start(out=out_d.ap(), in_=lg_t[:]).then_inc(sem_out, 16)
            sync.wait_ge(sem_out, 16)

        @block.tensor
        def _(tensor):
            tensor.wait_ge(sem_xoh, 16)
            for n in range(NPC):
                if n >= 2:
                    # PSUM slot n%2 free once all 4 products of row n-2 read it
                    tensor.wait_ge(sem_mul, 4 * n - 4)
                for jj in range(C // 512):
                    nc.tensor.matmul(
                        out=xrep[n % 2][:, jj * 512:(jj + 1) * 512],
                        lhsT=xoh_t[:, C + n * 128:C + (n + 1) * 128],
                        rhs=xoh_t[:, jj * 512:(jj + 1) * 512],
                        start=True, stop=True,
                    ).then_inc(sem_x, 1)

        @block.gpsimd
        def _(gpsimd):
            # NOTE: the HW indirect DMA consumes ONE offset per partition
            # (out[p, :] streams contiguously from M[offs[p]]), so each call
            # gathers exactly 128 rows. Per-slot semaphores + the sem_mul
            # throttle keep <=1 call in flight per semaphore so its 16
            # per-engine increments can't interleave with another call's.
            gpsimd.wait_ge(sem_idx, 16)
            for t in range(NT):
                if t >= GB:
                    gpsimd.wait_ge(sem_mul, t - GB + 1)
                base = (t % GB) * C
                if t == NT - 1:
                    # split the last tile along C so its product/reduce can
                    # start while the second half is still streaming
                    gpsimd.indirect_dma_start(
                        out=g_t[:, base:base + C // 2],
                        out_offset=None,
                        in_=m_d.ap(),
                        in_offset=bass.IndirectOffsetOnAxis(
                            ap=idx_t[:, t:t + 1], axis=0),
                    ).then_inc(sem_g[t % GB], 16)
                    gpsimd.indirect_dma_start(
                        out=g_t[:, base + C // 2:base + C],
                        out_offset=None,
                        in_=m_d.ap(),
                        in_offset=bass.IndirectOffsetOnAxis(
                            ap=idx_t[:, t:t + 1], axis=0),
                        element_offset=C // 2,
                    ).then_inc(sem_gl, 16)
                else:
                    gpsimd.indirect_dma_start(
                        out=g_t[:, base:base + C],
                        out_offset=None,
                        in_=m_d.ap(),
                        in_offset=bass.IndirectOffsetOnAxis(
                            ap=idx_t[:, t:t + 1], axis=0),
                    ).then_inc(sem_g[t % GB], 16)

        @block.vector
        def _(vector):
            for t in range(NT):
                n = t // 4
                if t % 4 == 0:
                    vector.wait_ge(sem_x, 4 * (n + 1))
                vector.wait_ge(sem_g[t % GB], 16 * (t // GB + 1))
                if t >= SB:
                    # scratch slot free once reduction t-SB retired
                    if _reducer_is_dve(t - SB):
                        vector.wait_ge(sem_rd, int(dve_count[t - SB]))
                    else:
                        vector.wait_ge(sem_ra, int(act_count[t - SB]))
                sc = sc_t[:, (t % SB) * C:(t % SB + 1) * C]
                gb = (t % GB) * C
                if t == NT - 1:
                    h = C // 2
                    nc.vector.tensor_tensor(
                        out=sc[:, :h], in0=g_t[:, gb:gb + h],
                        in1=xrep[n % 2][:, :h],
                        op=mybir.AluOpType.mult,
                    ).then_inc(sem_mul, 1)
                    vector.wait_ge(sem_gl, 16)
                    nc.vector.tensor_tensor(
                        out=sc[:, h:], in0=g_t[:, gb + h:gb + C],
                        in1=xrep[n % 2][:, h:],
                        op=mybir.AluOpType.mult,
                    ).then_inc(sem_mul, 1)
                else:
                    nc.vector.tensor_tensor(
                        out=sc, in0=g_t[:, gb:gb + C],
                        in1=xrep[n % 2][:],
                        op=mybir.AluOpType.mult,
                    ).then_inc(sem_mul, 1)
                if _reducer_is_dve(t):
                    # self-wait: the engine pipeline may not have drained the
                    # product write yet
                    vector.wait_ge(sem_mul, t + 1 + (1 if t == NT - 1 else 0))
                    nc.vector.tensor_reduce(
                        out=lg_t[:, t:t + 1], in_=sc,
                        axis=mybir.AxisListType.X,
                        op=mybir.AluOpType.add,
                    ).then_inc(sem_rd, 1)

        @block.scalar
        def _(scalar):
            for t in range(NT):
                if _reducer_is_dve(t):
                    continue
                scalar.wait_ge(sem_mul, t + 1)
                sc = sc_t[:, (t % SB) * C:(t % SB + 1) * C]
                # in-place: the mandatory elementwise out overwrites the
                # products we no longer need; only accum_out matters.
                nc.scalar.activation(
                    out=sc, in_=sc,
                    func=mybir.ActivationFunctionType.Copy,
                    accum_out=lg_t[:, t:t + 1],
                ).then_inc(sem_ra, 1)

    return nc


def _make_onehot():
    oh = np.zeros((NPC, NPC, 128), dtype=np.float32)
    for n in range(NPC):
        oh[n, n, :] = 1.0
    return oh.reshape(NPC, NPC * 128)


def make_in_maps(inputs, positive_index, negative_index, M):
    """Shard the full inputs into per-core input dicts."""
    index_list = np.concatenate(
        [np.asarray(positive_index), np.asarray(negative_index)], axis=1
    ).astype(np.int32)  # [N, K]
    inputs = np.ascontiguousarray(np.asarray(inputs, dtype=np.float32))
    M = np.ascontiguousarray(np.asarray(M, dtype=np.float32))
    oh = _make_onehot()

    in_maps = []
    for c in range(NCORES):
        idx_c = index_list[c * NPC:(c + 1) * NPC]  # [16, 512]
        # midx[p, n*JT + j] = idx_c[n, j*128 + p]
        midx = np.ascontiguousarray(
            idx_c.reshape(NPC, JT, 128).transpose(2, 0, 1).reshape(128, NT))
        in_maps.append({
            "xoh": np.ascontiguousarray(
                np.concatenate([inputs[c * NPC:(c + 1) * NPC], oh], axis=1)),
            "midx": midx,
            "mbank": M,
        })
    return in_maps


def assemble_logits(per_core_outs):
    """[128, 64] per core -> full [128, 512] logits."""
    shards = []
    for c in range(NCORES):
        out_c = per_core_outs[c]  # [128, NT]
        # logits[n, j*128+p] = out_c[p, n*JT+j]
        shard = out_c.reshape(128, NPC, JT).transpose(1, 2, 0).reshape(NPC, K)
        shards.append(shard)
    return np.concatenate(shards, axis=0).astype(np.float32)


def _loss_from_logits(logits, cof):
    logits = logits.astype(np.float32)
    m = logits.max(axis=1, keepdims=True)
    lse = m + np.log(np.exp(logits - m).sum(axis=1, keepdims=True))
    logp = logits - lse  # [N, K]
    loss = -(np.asarray(cof, dtype=np.float32) * logp[:, :KP].mean(axis=0)).sum()
    return np.float32(loss)


def kernel(inputs, positive_index, negative_index, cof, M):
    global _LAST_RESULTS
    in_maps = make_in_maps(inputs, positive_index, negative_index, M)
    nc = build_nc()
    trace = os.environ.get("KERNEL_TRACE", "0") == "1"
    res = run_bass_kernel_spmd(
        nc, in_maps, core_ids=list(range(NCORES)), trace=trace)
    _LAST_RESULTS = res
    logits = assemble_logits([r["logits_t"] for r in res.results])
    loss = _loss_from_logits(logits, cof)
    return loss, logits
